# revision 33
# baseline (speedup 1.0000x reference)
"""CGMM (Contextual Graph Markov Model) forward pass on 8 Trainium2 NeuronCores.

Self-contained: takes FULL inputs as numpy arrays, shards nodes/edges across
the 8 cores (graph parallel), runs a Bass/Tile kernel via
run_bass_kernel_spmd, returns the FULL [N, L, G] log-likelihood output.

Key layout (per core, nodes on partitions, cg = g*8 + c on free dim):
  layer 0:  u0[n, cg] = B0[c, x_n, g]*Pi[c, g]  via host-built one-hot(x) matmul
  layers 1..3:
            h split into NBANKS node-range banks; AllGather per bank overlaps
            the previous bank's compute (pipelined collectives)
            gather h_full_bank[src] per edge via dma_gather spread ROUND-ROBIN
            over 4 SWDGE queues (4 Q7 core pairs emit descriptors in parallel)
            aggT[cg, dst] = segment-sum via host-built one-hot matmuls
            (lhsT=gathered, rhs=onehot -> transposed aggregate, PSUM fp32),
            accumulated across banks in an SBUF fp32 tile
            QA^T = Qbig @ aggT; u = Bx * QA; Z = sum_c u; ll = log Z; h = u/Z
Host precomputes: edge sort/tiling, one-hot tiles (bf16), one-hot(x),
in-degree log-counts (applied as output post-processing: ll -= log cnt).
"""
import sys

sys.path.insert(0, "/opt/trn_rl_repo")

import numpy as np
import ml_dtypes

BF = ml_dtypes.bfloat16

# ---- problem sizes (hardcoded per contract) --------------------------------
N, E, C, M, G, L = 50000, 800000, 8, 32, 16, 4
NCORES = 8
CG = C * G  # 128
NBANKS = 3  # h banks (pipelined AllGather)
NQ = 4      # SWDGE queues used for dma_gather
TG = 16     # gather chunk size in 128-edge tiles


def split_blocks(nb, k):
    base = nb // k
    rem = nb % k
    return [base + (1 if i < rem else 0) for i in range(k)]


NSUB = 3  # AllGather sub-units per bank (table rows are sub-bank-major)


class Cfg:
    def __init__(self, n=N, e=E, ncores=NCORES):
        self.n = n
        self.e = e
        self.ncores = ncores
        self.npc = n // ncores
        self.nb = (self.npc + 127) // 128
        self.last_nn = self.npc - (self.nb - 1) * 128
        self.bank_blocks = split_blocks(self.nb, NBANKS)
        self.bank_first = np.concatenate([[0], np.cumsum(self.bank_blocks)])
        # nodes per bank within one core (last bank absorbs the short block)
        self.bank_node_start = [int(self.bank_first[k]) * 128
                                for k in range(NBANKS)]
        self.bank_nodes = [
            (int(self.bank_first[k + 1]) * 128 if k < NBANKS - 1 else self.npc)
            - self.bank_node_start[k]
            for k in range(NBANKS)]
        self.bank_of_block = np.searchsorted(
            self.bank_first[1:], np.arange(self.nb), side="right")
        # sub-banks (AG units) within each bank, aligned to blocks
        self.sub_blocks = [split_blocks(self.bank_blocks[k], NSUB)
                           for k in range(NBANKS)]
        # per bank: block index (within bank) where each sub starts,
        # node offset (within bank) and node count of each sub
        self.sub_first = [np.concatenate([[0], np.cumsum(self.sub_blocks[k])])
                          for k in range(NBANKS)]
        self.sub_node_start = [[int(self.sub_first[k][s]) * 128
                                for s in range(NSUB)] for k in range(NBANKS)]
        self.sub_nodes = [[
            (int(self.sub_first[k][s + 1]) * 128 if s < NSUB - 1
             else self.bank_nodes[k]) - self.sub_node_start[k][s]
            for s in range(NSUB)] for k in range(NBANKS)]


# ---- host preprocessing -----------------------------------------------------

def preprocess(x, edge_index, cfg):
    """Edge sort + tile schedule + one-hot tiles + degree counts (host)."""
    dst = np.asarray(edge_index[0], dtype=np.int64)
    src = np.asarray(edge_index[1], dtype=np.int64)
    x = np.asarray(x, dtype=np.int64)
    nc_, npc, nb = cfg.ncores, cfg.npc, cfg.nb

    owner = dst // npc
    per_core = []
    cnts = np.zeros((nc_, nb, NBANKS), dtype=np.int64)
    lncnt = np.zeros(cfg.n, dtype=np.float32)
    bob = cfg.bank_of_block
    bns = np.asarray(cfg.bank_node_start, dtype=np.int64)
    # per global block: bank, sub index, sub node-start/count (within bank)
    blk_bank = bob
    blk_sub = np.zeros(nb, dtype=np.int64)
    blk_sns = np.zeros(nb, dtype=np.int64)
    blk_snn = np.zeros(nb, dtype=np.int64)
    for bb in range(nb):
        k = int(bob[bb])
        bl = bb - int(cfg.bank_first[k])
        s = int(np.searchsorted(cfg.sub_first[k][1:], bl, side="right"))
        blk_sub[bb] = s
        blk_sns[bb] = cfg.sub_node_start[k][s]
        blk_snn[bb] = cfg.sub_nodes[k][s]
    for c in range(nc_):
        sel = owner == c
        d = dst[sel] - c * npc
        s = src[sel]
        deg = np.bincount(d, minlength=npc)
        lncnt[c * npc:(c + 1) * npc] = np.log(np.maximum(deg, 1))
        b = d // 128
        dl = d % 128
        sown = s // npc
        soff = s % npc
        sb = soff // 128
        kb = blk_bank[sb]
        # sub-bank-major table row: 8*sub_start + rank*sub_nodes + off_in_sub
        off_in_bank = soff - bns[kb]
        row = (nc_ * blk_sns[sb] + sown * blk_snn[sb]
               + (off_in_bank - blk_sns[sb]))
        per_core.append((b, dl, row, kb))
        key = b * NBANKS + kb
        cnts[c] = np.bincount(key, minlength=nb * NBANKS).reshape(nb, NBANKS)
    T = np.maximum(1, -(-cnts.max(axis=0) // 128))  # [nb, NBANKS]

    # tile offsets in (bank-major, block-minor) order
    off = np.zeros((NBANKS, nb), dtype=np.int64)
    pos = 0
    bank_t0 = []
    for k in range(NBANKS):
        bank_t0.append(pos)
        for b in range(nb):
            off[k, b] = pos
            pos += int(T[b, k])
    T_tot = pos
    bank_t1 = bank_t0[1:] + [T_tot]

    cores = []
    for c in range(nc_):
        b, dl, row, kb = per_core[c]
        idxs = np.zeros(T_tot * 128, dtype=np.int64)
        dls = np.full(T_tot * 128, -1, dtype=np.int64)
        order = np.argsort(kb * nb * 64 + b, kind="stable")
        b, dl, row, kb = b[order], dl[order], row[order], kb[order]
        # group boundaries: edges sorted by (bank, block)
        grp = kb * nb + b
        starts = np.searchsorted(grp, np.arange(NBANKS * nb), side="left")
        ends = np.searchsorted(grp, np.arange(NBANKS * nb), side="right")
        for k in range(NBANKS):
            for bb in range(nb):
                g0, g1 = starts[k * nb + bb], ends[k * nb + bb]
                if g1 <= g0:
                    continue
                a0 = off[k, bb] * 128
                idxs[a0:a0 + (g1 - g0)] = row[g0:g1]
                dls[a0:a0 + (g1 - g0)] = dl[g0:g1]

        # idx dram layout: [128, T_tot*8] int16; 16-row wrap, replicated 8x
        idx16 = idxs.astype(np.int16).reshape(-1, 16).T  # [16, T_tot*8]
        idx_d = np.tile(idx16, (8, 1))                   # [128, T_tot*8]

        # one-hot tiles: ohx[p, t*128 + d] = 1 iff dls[t*128+p] == d
        ohx = np.zeros((128, T_tot * 128), dtype=BF)
        i_all = np.arange(T_tot * 128)
        m = dls >= 0
        ohx[i_all[m] % 128, (i_all[m] // 128) * 128 + dls[m]] = 1

        # one-hot(x)^T: [32, nb*128] bf16
        xloc = np.zeros(nb * 128, dtype=np.int64)
        xloc[:npc] = x[c * npc:(c + 1) * npc]
        ohxt = np.zeros((M, nb * 128), dtype=BF)
        ohxt[xloc, np.arange(nb * 128)] = 1
        ohxt[:, npc:] = 0  # padded tail nodes: no contribution needed anyway

        cores.append({"idx": np.ascontiguousarray(idx_d),
                      "ohx": np.ascontiguousarray(ohx),
                      "ohxt": np.ascontiguousarray(ohxt)})
    sched = {"T": T, "off": off, "T_tot": T_tot,
             "bank_t0": bank_t0, "bank_t1": bank_t1}
    return cores, sched, lncnt


def permute_params(lambda_B0, lambda_Pi, lambda_Q, lambda_B):
    """Pure layout permutations (no compute): partition (g, c/k)-major views."""
    lamB0p = np.ascontiguousarray(
        np.transpose(np.asarray(lambda_B0, np.float32), (2, 0, 1)).reshape(G * C, M))
    lamPip = np.ascontiguousarray(np.asarray(lambda_Pi, np.float32).T)  # [G, C]
    lamQp = np.ascontiguousarray(
        np.transpose(np.asarray(lambda_Q, np.float32), (0, 3, 2, 1)).reshape(
            L - 1, G * C, C))
    lamBp = np.ascontiguousarray(
        np.transpose(np.asarray(lambda_B, np.float32), (0, 3, 1, 2)).reshape(
            L - 1, G * C, M))
    return {"lamB0p": lamB0p, "lamPip": lamPip, "lamQp": lamQp, "lamBp": lamBp}


def make_consts():
    ident_f = np.eye(128, dtype=np.float32)
    ident_b = np.eye(128, dtype=BF)
    pp = np.arange(128) // 8
    maskg = (pp[:, None] == pp[None, :]).astype(np.float32)
    return {"ident_f": ident_f, "ident_b": ident_b, "maskg": maskg}


# ---- bass kernel builder ----------------------------------------------------

def build_nc(cfg, sched):
    import concourse.bass as bass
    import concourse.bacc as bacc
    import concourse.mybir as mybir
    import concourse.tile as tile

    fp32 = mybir.dt.float32
    bf16 = mybir.dt.bfloat16
    i16 = mybir.dt.int16
    AX = mybir.AxisListType.X
    OP = mybir.AluOpType
    AF = mybir.ActivationFunctionType

    nb, npc, last_nn = cfg.nb, cfg.npc, cfg.last_nn
    T, off, T_tot = sched["T"], sched["off"], sched["T_tot"]
    bank_t0, bank_t1 = sched["bank_t0"], sched["bank_t1"]

    nc = bacc.Bacc("TRN2", target_bir_lowering=False, debug=False,
                   num_devices=cfg.ncores, num_swdge_queues=NQ)

    # ---- dram I/O
    idx_d = nc.dram_tensor("idx", [128, T_tot * 8], i16, kind="ExternalInput")
    ohx_d = nc.dram_tensor("ohx", [128, T_tot * 128], bf16, kind="ExternalInput")
    ohxt_d = nc.dram_tensor("ohxt", [M, nb * 128], bf16, kind="ExternalInput")
    lam_B0 = nc.dram_tensor("lamB0p", [128, M], fp32, kind="ExternalInput")
    lam_Pi = nc.dram_tensor("lamPip", [G, C], fp32, kind="ExternalInput")
    lam_Q = nc.dram_tensor("lamQp", [L - 1, 128, C], fp32, kind="ExternalInput")
    lam_B = nc.dram_tensor("lamBp", [L - 1, 128, M], fp32, kind="ExternalInput")
    pi_bounce = nc.dram_tensor("pi_bounce", [G * C], fp32)
    ident_f_d = nc.dram_tensor("ident_f", [128, 128], fp32, kind="ExternalInput")
    ident_b_d = nc.dram_tensor("ident_b", [128, 128], bf16, kind="ExternalInput")
    maskg_d = nc.dram_tensor("maskg", [128, 128], fp32, kind="ExternalInput")
    lls_d = nc.dram_tensor("lls", [npc, L * G], fp32, kind="ExternalOutput")

    h_slice = [[nc.dram_tensor(f"h_s{l}_{k}", [cfg.bank_nodes[k], CG], bf16)
                for k in range(NBANKS)] for l in range(L - 1)]
    h_full = [[nc.dram_tensor(f"h_f{l}_{k}", [cfg.ncores * cfg.bank_nodes[k], CG],
                              bf16, addr_space="Shared")
               for k in range(NBANKS)] for l in range(L - 1)]
    rgroups = [list(range(cfg.ncores))]

    # chunk table: list of (bank, t0, ntile); queue = index % NQ
    chunks = []
    tile2chunk = np.zeros(T_tot, dtype=np.int64)
    for k in range(NBANKS):
        for t0 in range(bank_t0[k], bank_t1[k], TG):
            ntile = min(TG, bank_t1[k] - t0)
            tile2chunk[t0:t0 + ntile] = len(chunks)
            chunks.append((k, t0, ntile))

    with tile.TileContext(nc) as tc:
        from contextlib import ExitStack
        with ExitStack() as ctx:
            res = ctx.enter_context(tc.tile_pool(name="res", bufs=1))
            sbp = ctx.enter_context(tc.tile_pool(name="sbp", bufs=3))
            gp = ctx.enter_context(tc.tile_pool(name="gp", bufs=3 * NQ))
            ohp = ctx.enter_context(tc.tile_pool(name="ohp", bufs=8))
            psp = ctx.enter_context(tc.tile_pool(name="psp", bufs=2, space="PSUM"))
            psa = ctx.enter_context(tc.tile_pool(name="psa", bufs=2, space="PSUM"))

            # ---- residents
            ident_f = res.tile([128, 128], fp32)
            nc.sync.dma_start(out=ident_f[:], in_=ident_f_d[:])
            ident_b = res.tile([128, 128], bf16)
            nc.sync.dma_start(out=ident_b[:], in_=ident_b_d[:])
            maskg = res.tile([128, 128], fp32)
            nc.sync.dma_start(out=maskg[:], in_=maskg_d[:])
            idx_t = res.tile([128, T_tot * 8], i16)
            nc.sync.dma_start(out=idx_t[:], in_=idx_d[:])
            ohxt = res.tile([M, nb * 128], bf16)
            nc.sync.dma_start(out=ohxt[:], in_=ohxt_d[:])
            out_sb = res.tile([128, nb * 64], fp32)   # lls accumulator
            # per-block bank-0 partial aggregates (bf16; re-injected into the
            # bank-1 PSUM chain via an identity matmul)
            aggb = [res.tile([128, 128], bf16, name=f"aggb{_b}")
                    for _b in range(nb)]
            qbig = res.tile([128, 128], bf16)
            barrT = res.tile([32, 128], bf16)         # layer's B table [m, cg]
            pi_col = res.tile([128, 1], fp32)

            # preload num_idxs registers once (avoids a MOVE per gather,
            # which eats Pool-engine dispatch slots)
            nidx_regs = {}
            for _k, _t0, _ntile in chunks:
                nval = _ntile * 128
                if nval not in nidx_regs:
                    nidx_regs[nval] = nc.gpsimd.to_reg(nval)

            def softmax_free(raw, nfree, tag):
                mx = sbp.tile([raw.shape[0], 1], fp32, tag=f"{tag}mx")
                nc.vector.tensor_reduce(out=mx[:], in_=raw[:], axis=AX,
                                        op=OP.max, negate=True)
                ex = sbp.tile([raw.shape[0], nfree], fp32, tag=f"{tag}ex")
                nc.scalar.activation(out=ex[:], in_=raw[:], func=AF.Exp,
                                     bias=mx[:, 0:1], scale=1.0)
                sm = sbp.tile([raw.shape[0], 1], fp32, tag=f"{tag}sm")
                nc.vector.reduce_sum(out=sm[:], in_=ex[:], axis=AX)
                rs = sbp.tile([raw.shape[0], 1], fp32, tag=f"{tag}rs")
                nc.vector.reciprocal(out=rs[:], in_=sm[:])
                out = sbp.tile([raw.shape[0], nfree], fp32, tag=f"{tag}out")
                nc.vector.tensor_scalar(out=out[:], in0=ex[:], scalar1=rs[:, 0:1],
                                        scalar2=None, op0=OP.mult)
                return out

            def transpose_to(dest_sb, src_sb, pdim, fdim):
                ps = psp.tile([fdim, pdim], fp32, tag="trp", space="PSUM")
                nc.tensor.transpose(out=ps[:], in_=src_sb[:],
                                    identity=ident_f[:pdim, :pdim])
                nc.scalar.copy(out=dest_sb[:], in_=ps[:])

            def bank_of_block(b):
                return int(cfg.bank_of_block[b])

            max_bb = max(cfg.bank_blocks)
            hb_cur = {}  # bank -> SBUF tile accumulating this layer's h bank

            def emit_h(l, b, u_ap, Z):
                """rz = 1/Z; h = u*rz (bf16) into bank SBUF tile; at bank end
                one batched DMA to h_slice + fire the AllGather. u_ap: AP."""
                kh = bank_of_block(b)
                bl = b - int(cfg.bank_first[kh])
                if bl == 0:
                    hb_cur[kh] = sbp.tile([128, max_bb * 128], bf16, tag="hb",
                                          name="hbank")
                hb = hb_cur[kh]
                rz = sbp.tile([128, G], fp32, tag="rz")
                nc.vector.reciprocal(out=rz[:], in_=Z[:])
                nc.vector.tensor_tensor(
                    out=hb[:, bl * 128:(bl + 1) * 128].rearrange(
                        "p (g c) -> p g c", c=C),
                    in0=u_ap.rearrange("p (g c) -> p g c", c=C),
                    in1=rz[:].to_broadcast([128, G, C]), op=OP.mult)
                # at each sub-bank boundary: batched DMA + sub AllGather
                sub = int(np.searchsorted(cfg.sub_first[kh][1:], bl,
                                          side="right"))
                if bl == int(cfg.sub_first[kh][sub + 1]) - 1:
                    sns = cfg.sub_node_start[kh][sub]
                    snn = cfg.sub_nodes[kh][sub]
                    # ACT-engine HWDGE: keeps the Sync FIFO free for oh loads
                    if b < nb - 1:  # no short block in this sub
                        nc.scalar.dma_start(
                            out=h_slice[l][kh][sns:sns + snn, :].rearrange(
                                "(b p) d -> p b d", p=128),
                            in_=hb[:, sns:sns + snn].rearrange(
                                "p (b d) -> p b d", d=128))
                    else:
                        full = snn // 128  # whole blocks; short block last
                        if full > 0:
                            nc.scalar.dma_start(
                                out=h_slice[l][kh][sns:sns + full * 128, :]
                                .rearrange("(b p) d -> p b d", p=128),
                                in_=hb[:, sns:sns + full * 128].rearrange(
                                    "p (b d) -> p b d", d=128))
                        nc.scalar.dma_start(
                            out=h_slice[l][kh][sns + full * 128:sns + snn, :],
                            in_=hb[:last_nn,
                                   sns + full * 128:sns + full * 128 + 128])
                    nc.gpsimd.collective_compute(
                        "AllGather", OP.bypass, replica_groups=rgroups,
                        ins=[h_slice[l][kh][sns:sns + snn, :]],
                        outs=[h_full[l][kh][cfg.ncores * sns:
                                            cfg.ncores * (sns + snn), :]])

            # ================= layer 0 =================
            braw = sbp.tile([128, M], fp32, tag="braw")
            nc.sync.dma_start(out=braw[:], in_=lam_B0[:])
            b0sm = softmax_free(braw, M, "b")
            praw = sbp.tile([G, C], fp32, tag="praw")
            nc.sync.dma_start(out=praw[:], in_=lam_Pi[:])
            pism = softmax_free(praw, C, "p")
            nc.sync.dma_start(out=pi_bounce[:].rearrange("(g c) -> g c", c=C),
                              in_=pism[:])
            nc.sync.dma_start(out=pi_col[:], in_=pi_bounce[:, None])
            b0p = sbp.tile([128, M], fp32, tag="b0p")
            nc.vector.tensor_scalar(out=b0p[:], in0=b0sm[:], scalar1=pi_col[:, 0:1],
                                    scalar2=None, op0=OP.mult)
            transpose_to(barrT, b0p, 128, 32)

            l0_ps = {}
            for i in range(nb + 1):
                if i < nb:
                    u0p = psp.tile([128, 128], fp32,
                                   tag="bx" if i % 2 else "qa", space="PSUM")
                    nc.tensor.matmul(out=u0p[:],
                                     lhsT=ohxt[:, i * 128:(i + 1) * 128],
                                     rhs=barrT[:], start=True, stop=True)
                    l0_ps[i] = u0p
                if i >= 1:
                    b = i - 1
                    ps = l0_ps.pop(b)
                    Z = sbp.tile([128, G], fp32, tag="Z")
                    nc.vector.reduce_sum(out=Z[:], in_=ps[:].rearrange(
                        "p (g c) -> p g c", c=C), axis=AX)
                    nc.scalar.activation(out=out_sb[:, b * 64:b * 64 + G],
                                         in_=Z[:], func=AF.Ln)
                    emit_h(0, b, ps[:], Z)

            # ================= graph layers =================
            for l in range(1, L):
                lq = l - 1

                qraw = sbp.tile([128, C], fp32, tag="qraw")
                nc.sync.dma_start(out=qraw[:], in_=lam_Q[lq])
                qsm = softmax_free(qraw, C, "q")  # [(g k), c]
                qsm_ap = qsm[:]
                qsm_bc = bass.AP(qsm_ap.tensor, qsm_ap.offset,
                                 [qsm_ap.ap[0], [0, G], qsm_ap.ap[1]])
                nc.vector.tensor_tensor(
                    out=qbig[:].rearrange("p (g c) -> p g c", c=C),
                    in0=qsm_bc,
                    in1=maskg[:].rearrange("p (g c) -> p g c", c=C),
                    op=OP.mult)
                braw2 = sbp.tile([128, M], fp32, tag="braw")
                nc.sync.dma_start(out=braw2[:], in_=lam_B[lq])
                bsm = softmax_free(braw2, M, "b")
                transpose_to(barrT, bsm, 128, 32)

                # ---- chunk caches (fresh per layer)
                gat_cache = {}
                oh_cache = {}

                def get_gat(gt, lq=lq):
                    ci = int(tile2chunk[gt])
                    if ci not in gat_cache:
                        k, t0, ntile = chunks[ci]
                        buf = gp.tile([128, ntile * 128], bf16, tag="g")
                        nc.gpsimd.dma_gather(
                            out_ap=buf[:].rearrange("p (t e) -> p t e", e=128),
                            in_ap=h_full[lq][k][:],
                            idxs_ap=idx_t[:, t0 * 8:(t0 + ntile) * 8],
                            num_idxs=ntile * 128,
                            num_idxs_reg=nidx_regs[ntile * 128],
                            elem_size=128,
                            single_packet=False,
                            queue_num=ci % NQ)
                        gat_cache[ci] = buf
                    k, t0, ntile = chunks[ci]
                    return gat_cache[ci][:].rearrange(
                        "p (t e) -> p t e", e=128)[:, gt - t0, :]

                def get_oh(gt):
                    ci = int(tile2chunk[gt])
                    if ci not in oh_cache:
                        k, t0, ntile = chunks[ci]
                        buf = ohp.tile([128, ntile * 128], bf16, tag="oh")
                        nc.sync.dma_start(
                            out=buf[:], in_=ohx_d[:, t0 * 128:(t0 + ntile) * 128])
                        oh_cache[ci] = buf
                    k, t0, ntile = chunks[ci]
                    return oh_cache[ci][:, (gt - t0) * 128:(gt - t0 + 1) * 128]

                # ---- sweep banks 0..NBANKS-2 -> bf16 partial aggregates
                for k in range(NBANKS - 1):
                    for b in range(nb):
                        nt = int(T[b, k])
                        ps = psa.tile([128, 128], fp32, tag="agg", space="PSUM")
                        if k > 0:  # re-inject prior partial: I^T @ aggb = aggb
                            nc.tensor.matmul(out=ps[:], lhsT=ident_b[:],
                                             rhs=aggb[b][:], start=True,
                                             stop=False)
                        for t in range(nt):
                            gt = int(off[k, b]) + t
                            nc.tensor.matmul(out=ps[:], lhsT=get_gat(gt),
                                             rhs=get_oh(gt),
                                             start=(t == 0 and k == 0),
                                             stop=(t == nt - 1))
                        nc.scalar.copy(out=aggb[b][:], in_=ps[:])

                # ---- last bank: block-major, 3-stage pipelined finish
                kL = NBANKS - 1
                aggF_d = {}
                s1_d = {}
                for i in range(nb + 2):
                    if i < nb:
                        b = i
                        nt = int(T[b, kL])
                        ps = psa.tile([128, 128], fp32, tag="agg", space="PSUM")
                        # re-inject partial: I^T @ aggb = aggb
                        nc.tensor.matmul(out=ps[:], lhsT=ident_b[:],
                                         rhs=aggb[b][:], start=True, stop=False)
                        for t in range(nt):
                            gt = int(off[kL, b]) + t
                            nc.tensor.matmul(out=ps[:], lhsT=get_gat(gt),
                                             rhs=get_oh(gt),
                                             start=False, stop=(t == nt - 1))
                        aggF = sbp.tile([128, 128], bf16, tag="aggF")
                        nc.scalar.copy(out=aggF[:], in_=ps[:])
                        aggF_d[b] = aggF
                    if 1 <= i <= nb:
                        b = i - 1
                        qaT = psp.tile([128, 128], fp32, tag="qa", space="PSUM")
                        nc.tensor.matmul(out=qaT[:], lhsT=qbig[:],
                                         rhs=aggF_d.pop(b)[:],
                                         start=True, stop=True)
                        qaTsb = sbp.tile([128, 128], fp32, tag="qaTsb")
                        nc.scalar.copy(out=qaTsb[:], in_=qaT[:])
                        bx = psp.tile([128, 128], fp32, tag="bx", space="PSUM")
                        nc.tensor.matmul(out=bx[:],
                                         lhsT=ohxt[:, b * 128:(b + 1) * 128],
                                         rhs=barrT[:], start=True, stop=True)
                        bxsb = sbp.tile([128, 128], fp32, tag="bxsb")
                        nc.scalar.copy(out=bxsb[:], in_=bx[:])
                        s1_d[b] = (qaTsb, bxsb)
                    if 2 <= i:
                        b = i - 2
                        qaTsb, bxsb = s1_d.pop(b)
                        qa2 = psp.tile([128, 128], fp32, tag="trp", space="PSUM")
                        nc.tensor.transpose(out=qa2[:], in_=qaTsb[:],
                                            identity=ident_f[:])
                        u = sbp.tile([128, 128], fp32, tag="u")
                        nc.vector.tensor_tensor(out=u[:], in0=qa2[:], in1=bxsb[:],
                                                op=OP.mult)
                        Z = sbp.tile([128, G], fp32, tag="Z")
                        nc.vector.reduce_sum(out=Z[:], in_=u[:].rearrange(
                            "p (g c) -> p g c", c=C), axis=AX)
                        nc.scalar.activation(
                            out=out_sb[:, b * 64 + l * G:b * 64 + (l + 1) * G],
                            in_=Z[:], func=AF.Ln)
                        if l < L - 1:
                            emit_h(l, b, u[:], Z)

            # ---- write lls out
            if nb > 1:
                nc.sync.dma_start(
                    out=lls_d[:(nb - 1) * 128, :].rearrange(
                        "(b p) c -> p b c", p=128),
                    in_=out_sb[:].rearrange("p (b c) -> p b c", c=64)[:, :nb - 1, :])
            nc.sync.dma_start(
                out=lls_d[(nb - 1) * 128:, :],
                in_=out_sb[:last_nn, (nb - 1) * 64:nb * 64])

    nc.compile()
    return nc


# ---- entry point ------------------------------------------------------------

def kernel(x, edge_index, lambda_B0, lambda_Pi, lambda_Q, lambda_B):
    cfg = Cfg()
    cores, sched, lncnt = preprocess(x, edge_index, cfg)
    consts = make_consts()
    nc = build_nc(cfg, sched)

    from concourse.bass_utils import run_bass_kernel_spmd
    params = permute_params(lambda_B0, lambda_Pi, lambda_Q, lambda_B)
    in_maps = []
    for c in range(cfg.ncores):
        m = dict(cores[c])
        m.update(params)
        m.update({k: np.ascontiguousarray(v) for k, v in consts.items()})
        in_maps.append(m)

    res = run_bass_kernel_spmd(nc, in_maps, core_ids=list(range(cfg.ncores)))
    out = np.concatenate([res.results[c]["lls"] for c in range(cfg.ncores)],
                         axis=0).reshape(N, L, G).astype(np.float32)
    out[:, 1:, :] -= lncnt[:, None, None]
    return out


# revision 34
# speedup vs baseline: 1.1027x; 1.1027x over previous
"""CGMM (Contextual Graph Markov Model) forward pass on 8 Trainium2 NeuronCores.

Self-contained: takes FULL inputs as numpy arrays, shards nodes/edges across
the 8 cores (graph parallel), runs a Bass/Tile kernel via
run_bass_kernel_spmd, returns the FULL [N, L, G] log-likelihood output.

Key layout (per core, nodes on partitions, cg = g*8 + c on free dim):
  layer 0:  u0[n, cg] = B0[c, x_n, g]*Pi[c, g]  via host-built one-hot(x) matmul
  layers 1..3:
            h split into NBANKS node-range banks; AllGather per bank overlaps
            the previous bank's compute (pipelined collectives)
            gather h_full_bank[src] per edge via dma_gather spread ROUND-ROBIN
            over 4 SWDGE queues (4 Q7 core pairs emit descriptors in parallel)
            aggT[cg, dst] = segment-sum via host-built one-hot matmuls
            (lhsT=gathered, rhs=onehot -> transposed aggregate, PSUM fp32),
            accumulated across banks in an SBUF fp32 tile
            QA^T = Qbig @ aggT; u = Bx * QA; Z = sum_c u; ll = log Z; h = u/Z
Host precomputes: edge sort/tiling, one-hot tiles (bf16), one-hot(x),
in-degree log-counts (applied as output post-processing: ll -= log cnt).
"""
import sys

sys.path.insert(0, "/opt/trn_rl_repo")

import numpy as np
import ml_dtypes

BF = ml_dtypes.bfloat16

# ---- problem sizes (hardcoded per contract) --------------------------------
N, E, C, M, G, L = 50000, 800000, 8, 32, 16, 4
NCORES = 8
CG = C * G  # 128
NBANKS = 2  # h banks (pipelined AllGather)
NQ = 4      # SWDGE queues used for dma_gather
TG = 16     # gather chunk size in 128-edge tiles


def split_blocks(nb, k):
    base = nb // k
    rem = nb % k
    return [base + (1 if i < rem else 0) for i in range(k)]


NSUB = 3  # AllGather sub-units per bank (table rows are sub-bank-major)


class Cfg:
    def __init__(self, n=N, e=E, ncores=NCORES):
        self.n = n
        self.e = e
        self.ncores = ncores
        self.npc = n // ncores
        self.nb = (self.npc + 127) // 128
        self.last_nn = self.npc - (self.nb - 1) * 128
        self.bank_blocks = split_blocks(self.nb, NBANKS)
        self.bank_first = np.concatenate([[0], np.cumsum(self.bank_blocks)])
        # nodes per bank within one core (last bank absorbs the short block)
        self.bank_node_start = [int(self.bank_first[k]) * 128
                                for k in range(NBANKS)]
        self.bank_nodes = [
            (int(self.bank_first[k + 1]) * 128 if k < NBANKS - 1 else self.npc)
            - self.bank_node_start[k]
            for k in range(NBANKS)]
        self.bank_of_block = np.searchsorted(
            self.bank_first[1:], np.arange(self.nb), side="right")
        # sub-banks (AG units) within each bank, aligned to blocks
        self.sub_blocks = [split_blocks(self.bank_blocks[k], NSUB)
                           for k in range(NBANKS)]
        # per bank: block index (within bank) where each sub starts,
        # node offset (within bank) and node count of each sub
        self.sub_first = [np.concatenate([[0], np.cumsum(self.sub_blocks[k])])
                          for k in range(NBANKS)]
        self.sub_node_start = [[int(self.sub_first[k][s]) * 128
                                for s in range(NSUB)] for k in range(NBANKS)]
        self.sub_nodes = [[
            (int(self.sub_first[k][s + 1]) * 128 if s < NSUB - 1
             else self.bank_nodes[k]) - self.sub_node_start[k][s]
            for s in range(NSUB)] for k in range(NBANKS)]


# ---- host preprocessing -----------------------------------------------------

def preprocess(x, edge_index, cfg):
    """Edge sort + tile schedule + one-hot tiles + degree counts (host)."""
    dst = np.asarray(edge_index[0], dtype=np.int64)
    src = np.asarray(edge_index[1], dtype=np.int64)
    x = np.asarray(x, dtype=np.int64)
    nc_, npc, nb = cfg.ncores, cfg.npc, cfg.nb

    owner = dst // npc
    per_core = []
    cnts = np.zeros((nc_, nb, NBANKS), dtype=np.int64)
    lncnt = np.zeros(cfg.n, dtype=np.float32)
    bob = cfg.bank_of_block
    bns = np.asarray(cfg.bank_node_start, dtype=np.int64)
    # per global block: bank, sub index, sub node-start/count (within bank)
    blk_bank = bob
    blk_sub = np.zeros(nb, dtype=np.int64)
    blk_sns = np.zeros(nb, dtype=np.int64)
    blk_snn = np.zeros(nb, dtype=np.int64)
    for bb in range(nb):
        k = int(bob[bb])
        bl = bb - int(cfg.bank_first[k])
        s = int(np.searchsorted(cfg.sub_first[k][1:], bl, side="right"))
        blk_sub[bb] = s
        blk_sns[bb] = cfg.sub_node_start[k][s]
        blk_snn[bb] = cfg.sub_nodes[k][s]
    for c in range(nc_):
        sel = owner == c
        d = dst[sel] - c * npc
        s = src[sel]
        deg = np.bincount(d, minlength=npc)
        lncnt[c * npc:(c + 1) * npc] = np.log(np.maximum(deg, 1))
        b = d // 128
        dl = d % 128
        sown = s // npc
        soff = s % npc
        sb = soff // 128
        kb = blk_bank[sb]
        # sub-bank-major table row: 8*sub_start + rank*sub_nodes + off_in_sub
        off_in_bank = soff - bns[kb]
        row = (nc_ * blk_sns[sb] + sown * blk_snn[sb]
               + (off_in_bank - blk_sns[sb]))
        per_core.append((b, dl, row, kb))
        key = b * NBANKS + kb
        cnts[c] = np.bincount(key, minlength=nb * NBANKS).reshape(nb, NBANKS)
    T = np.maximum(1, -(-cnts.max(axis=0) // 128))  # [nb, NBANKS]

    # tile offsets in (bank-major, block-minor) order
    off = np.zeros((NBANKS, nb), dtype=np.int64)
    pos = 0
    bank_t0 = []
    for k in range(NBANKS):
        bank_t0.append(pos)
        for b in range(nb):
            off[k, b] = pos
            pos += int(T[b, k])
    T_tot = pos
    bank_t1 = bank_t0[1:] + [T_tot]

    cores = []
    for c in range(nc_):
        b, dl, row, kb = per_core[c]
        idxs = np.zeros(T_tot * 128, dtype=np.int64)
        dls = np.full(T_tot * 128, -1, dtype=np.int64)
        order = np.argsort(kb * nb * 64 + b, kind="stable")
        b, dl, row, kb = b[order], dl[order], row[order], kb[order]
        # group boundaries: edges sorted by (bank, block)
        grp = kb * nb + b
        starts = np.searchsorted(grp, np.arange(NBANKS * nb), side="left")
        ends = np.searchsorted(grp, np.arange(NBANKS * nb), side="right")
        for k in range(NBANKS):
            for bb in range(nb):
                g0, g1 = starts[k * nb + bb], ends[k * nb + bb]
                if g1 <= g0:
                    continue
                a0 = off[k, bb] * 128
                idxs[a0:a0 + (g1 - g0)] = row[g0:g1]
                dls[a0:a0 + (g1 - g0)] = dl[g0:g1]

        # idx dram layout: [128, T_tot*8] int16; 16-row wrap, replicated 8x
        idx16 = idxs.astype(np.int16).reshape(-1, 16).T  # [16, T_tot*8]
        idx_d = np.tile(idx16, (8, 1))                   # [128, T_tot*8]

        # one-hot tiles: ohx[p, t*128 + d] = 1 iff dls[t*128+p] == d
        ohx = np.zeros((128, T_tot * 128), dtype=BF)
        i_all = np.arange(T_tot * 128)
        m = dls >= 0
        ohx[i_all[m] % 128, (i_all[m] // 128) * 128 + dls[m]] = 1

        # one-hot(x)^T: [32, nb*128] bf16
        xloc = np.zeros(nb * 128, dtype=np.int64)
        xloc[:npc] = x[c * npc:(c + 1) * npc]
        ohxt = np.zeros((M, nb * 128), dtype=BF)
        ohxt[xloc, np.arange(nb * 128)] = 1
        ohxt[:, npc:] = 0  # padded tail nodes: no contribution needed anyway

        cores.append({"idx": np.ascontiguousarray(idx_d),
                      "ohx": np.ascontiguousarray(ohx),
                      "ohxt": np.ascontiguousarray(ohxt)})
    sched = {"T": T, "off": off, "T_tot": T_tot,
             "bank_t0": bank_t0, "bank_t1": bank_t1}
    return cores, sched, lncnt


def permute_params(lambda_B0, lambda_Pi, lambda_Q, lambda_B):
    """Pure layout permutations (no compute): partition (g, c/k)-major views."""
    lamB0p = np.ascontiguousarray(
        np.transpose(np.asarray(lambda_B0, np.float32), (2, 0, 1)).reshape(G * C, M))
    lamPip = np.ascontiguousarray(np.asarray(lambda_Pi, np.float32).T)  # [G, C]
    lamQp = np.ascontiguousarray(
        np.transpose(np.asarray(lambda_Q, np.float32), (0, 3, 2, 1)).reshape(
            L - 1, G * C, C))
    lamBp = np.ascontiguousarray(
        np.transpose(np.asarray(lambda_B, np.float32), (0, 3, 1, 2)).reshape(
            L - 1, G * C, M))
    return {"lamB0p": lamB0p, "lamPip": lamPip, "lamQp": lamQp, "lamBp": lamBp}


def make_consts():
    ident_f = np.eye(128, dtype=np.float32)
    ident_b = np.eye(128, dtype=BF)
    pp = np.arange(128) // 8
    maskg = (pp[:, None] == pp[None, :]).astype(np.float32)
    return {"ident_f": ident_f, "ident_b": ident_b, "maskg": maskg}


# ---- bass kernel builder ----------------------------------------------------

def build_nc(cfg, sched):
    import concourse.bass as bass
    import concourse.bacc as bacc
    import concourse.mybir as mybir
    import concourse.tile as tile

    fp32 = mybir.dt.float32
    bf16 = mybir.dt.bfloat16
    i16 = mybir.dt.int16
    AX = mybir.AxisListType.X
    OP = mybir.AluOpType
    AF = mybir.ActivationFunctionType

    nb, npc, last_nn = cfg.nb, cfg.npc, cfg.last_nn
    T, off, T_tot = sched["T"], sched["off"], sched["T_tot"]
    bank_t0, bank_t1 = sched["bank_t0"], sched["bank_t1"]

    nc = bacc.Bacc("TRN2", target_bir_lowering=False, debug=False,
                   num_devices=cfg.ncores, num_swdge_queues=NQ)

    # ---- dram I/O
    idx_d = nc.dram_tensor("idx", [128, T_tot * 8], i16, kind="ExternalInput")
    ohx_d = nc.dram_tensor("ohx", [128, T_tot * 128], bf16, kind="ExternalInput")
    ohxt_d = nc.dram_tensor("ohxt", [M, nb * 128], bf16, kind="ExternalInput")
    lam_B0 = nc.dram_tensor("lamB0p", [128, M], fp32, kind="ExternalInput")
    lam_Pi = nc.dram_tensor("lamPip", [G, C], fp32, kind="ExternalInput")
    lam_Q = nc.dram_tensor("lamQp", [L - 1, 128, C], fp32, kind="ExternalInput")
    lam_B = nc.dram_tensor("lamBp", [L - 1, 128, M], fp32, kind="ExternalInput")
    pi_bounce = nc.dram_tensor("pi_bounce", [G * C], fp32)
    ident_f_d = nc.dram_tensor("ident_f", [128, 128], fp32, kind="ExternalInput")
    ident_b_d = nc.dram_tensor("ident_b", [128, 128], bf16, kind="ExternalInput")
    maskg_d = nc.dram_tensor("maskg", [128, 128], fp32, kind="ExternalInput")
    lls_d = nc.dram_tensor("lls", [npc, L * G], fp32, kind="ExternalOutput")

    h_slice = [[nc.dram_tensor(f"h_s{l}_{k}", [cfg.bank_nodes[k], CG], bf16)
                for k in range(NBANKS)] for l in range(L - 1)]
    h_full = [[nc.dram_tensor(f"h_f{l}_{k}", [cfg.ncores * cfg.bank_nodes[k], CG],
                              bf16, addr_space="Shared")
               for k in range(NBANKS)] for l in range(L - 1)]
    rgroups = [list(range(cfg.ncores))]

    # chunk table: list of (bank, t0, ntile); queue = index % NQ
    chunks = []
    tile2chunk = np.zeros(T_tot, dtype=np.int64)
    for k in range(NBANKS):
        for t0 in range(bank_t0[k], bank_t1[k], TG):
            ntile = min(TG, bank_t1[k] - t0)
            tile2chunk[t0:t0 + ntile] = len(chunks)
            chunks.append((k, t0, ntile))

    with tile.TileContext(nc) as tc:
        from contextlib import ExitStack
        with ExitStack() as ctx:
            res = ctx.enter_context(tc.tile_pool(name="res", bufs=1))
            sbp = ctx.enter_context(tc.tile_pool(name="sbp", bufs=3))
            gp = ctx.enter_context(tc.tile_pool(name="gp", bufs=4 * NQ))
            ohp = ctx.enter_context(tc.tile_pool(name="ohp", bufs=10))
            psp = ctx.enter_context(tc.tile_pool(name="psp", bufs=2, space="PSUM"))
            psa = ctx.enter_context(tc.tile_pool(name="psa", bufs=2, space="PSUM"))

            # ---- residents
            ident_f = res.tile([128, 128], fp32)
            nc.sync.dma_start(out=ident_f[:], in_=ident_f_d[:])
            ident_b = res.tile([128, 128], bf16)
            nc.sync.dma_start(out=ident_b[:], in_=ident_b_d[:])
            maskg = res.tile([128, 128], fp32)
            nc.sync.dma_start(out=maskg[:], in_=maskg_d[:])
            idx_t = res.tile([128, T_tot * 8], i16)
            nc.sync.dma_start(out=idx_t[:], in_=idx_d[:])
            ohxt = res.tile([M, nb * 128], bf16)
            nc.sync.dma_start(out=ohxt[:], in_=ohxt_d[:])
            out_sb = res.tile([128, nb * 64], fp32)   # lls accumulator
            # per-block bank-0 partial aggregates (bf16; re-injected into the
            # bank-1 PSUM chain via an identity matmul)
            aggb = [res.tile([128, 128], bf16, name=f"aggb{_b}")
                    for _b in range(nb)]
            qbig = res.tile([128, 128], bf16)
            barrT = res.tile([32, 128], bf16)         # layer's B table [m, cg]
            pi_col = res.tile([128, 1], fp32)

            # preload num_idxs registers once (avoids a MOVE per gather,
            # which eats Pool-engine dispatch slots)
            nidx_regs = {}
            for _k, _t0, _ntile in chunks:
                nval = _ntile * 128
                if nval not in nidx_regs:
                    nidx_regs[nval] = nc.gpsimd.to_reg(nval)

            def softmax_free(raw, nfree, tag):
                mx = sbp.tile([raw.shape[0], 1], fp32, tag=f"{tag}mx")
                nc.vector.tensor_reduce(out=mx[:], in_=raw[:], axis=AX,
                                        op=OP.max, negate=True)
                ex = sbp.tile([raw.shape[0], nfree], fp32, tag=f"{tag}ex")
                nc.scalar.activation(out=ex[:], in_=raw[:], func=AF.Exp,
                                     bias=mx[:, 0:1], scale=1.0)
                sm = sbp.tile([raw.shape[0], 1], fp32, tag=f"{tag}sm")
                nc.vector.reduce_sum(out=sm[:], in_=ex[:], axis=AX)
                rs = sbp.tile([raw.shape[0], 1], fp32, tag=f"{tag}rs")
                nc.vector.reciprocal(out=rs[:], in_=sm[:])
                out = sbp.tile([raw.shape[0], nfree], fp32, tag=f"{tag}out")
                nc.vector.tensor_scalar(out=out[:], in0=ex[:], scalar1=rs[:, 0:1],
                                        scalar2=None, op0=OP.mult)
                return out

            def transpose_to(dest_sb, src_sb, pdim, fdim):
                ps = psp.tile([fdim, pdim], fp32, tag="trp", space="PSUM")
                nc.tensor.transpose(out=ps[:], in_=src_sb[:],
                                    identity=ident_f[:pdim, :pdim])
                nc.scalar.copy(out=dest_sb[:], in_=ps[:])

            def bank_of_block(b):
                return int(cfg.bank_of_block[b])

            max_bb = max(cfg.bank_blocks)
            hb_cur = {}  # bank -> SBUF tile accumulating this layer's h bank

            def emit_h(l, b, u_ap, Z):
                """rz = 1/Z; h = u*rz (bf16) into bank SBUF tile; at bank end
                one batched DMA to h_slice + fire the AllGather. u_ap: AP."""
                kh = bank_of_block(b)
                bl = b - int(cfg.bank_first[kh])
                if bl == 0:
                    hb_cur[kh] = sbp.tile([128, max_bb * 128], bf16, tag="hb",
                                          name="hbank")
                hb = hb_cur[kh]
                rz = sbp.tile([128, G], fp32, tag="rz")
                nc.vector.reciprocal(out=rz[:], in_=Z[:])
                nc.vector.tensor_tensor(
                    out=hb[:, bl * 128:(bl + 1) * 128].rearrange(
                        "p (g c) -> p g c", c=C),
                    in0=u_ap.rearrange("p (g c) -> p g c", c=C),
                    in1=rz[:].to_broadcast([128, G, C]), op=OP.mult)
                # at each sub-bank boundary: batched DMA + sub AllGather
                sub = int(np.searchsorted(cfg.sub_first[kh][1:], bl,
                                          side="right"))
                if bl == int(cfg.sub_first[kh][sub + 1]) - 1:
                    sns = cfg.sub_node_start[kh][sub]
                    snn = cfg.sub_nodes[kh][sub]
                    # ACT-engine HWDGE: keeps the Sync FIFO free for oh loads
                    if b < nb - 1:  # no short block in this sub
                        nc.scalar.dma_start(
                            out=h_slice[l][kh][sns:sns + snn, :].rearrange(
                                "(b p) d -> p b d", p=128),
                            in_=hb[:, sns:sns + snn].rearrange(
                                "p (b d) -> p b d", d=128))
                    else:
                        full = snn // 128  # whole blocks; short block last
                        if full > 0:
                            nc.scalar.dma_start(
                                out=h_slice[l][kh][sns:sns + full * 128, :]
                                .rearrange("(b p) d -> p b d", p=128),
                                in_=hb[:, sns:sns + full * 128].rearrange(
                                    "p (b d) -> p b d", d=128))
                        nc.scalar.dma_start(
                            out=h_slice[l][kh][sns + full * 128:sns + snn, :],
                            in_=hb[:last_nn,
                                   sns + full * 128:sns + full * 128 + 128])
                    nc.gpsimd.collective_compute(
                        "AllGather", OP.bypass, replica_groups=rgroups,
                        ins=[h_slice[l][kh][sns:sns + snn, :]],
                        outs=[h_full[l][kh][cfg.ncores * sns:
                                            cfg.ncores * (sns + snn), :]])

            # ================= layer 0 =================
            braw = sbp.tile([128, M], fp32, tag="braw")
            nc.sync.dma_start(out=braw[:], in_=lam_B0[:])
            b0sm = softmax_free(braw, M, "b")
            praw = sbp.tile([G, C], fp32, tag="praw")
            nc.sync.dma_start(out=praw[:], in_=lam_Pi[:])
            pism = softmax_free(praw, C, "p")
            nc.sync.dma_start(out=pi_bounce[:].rearrange("(g c) -> g c", c=C),
                              in_=pism[:])
            nc.sync.dma_start(out=pi_col[:], in_=pi_bounce[:, None])
            b0p = sbp.tile([128, M], fp32, tag="b0p")
            nc.vector.tensor_scalar(out=b0p[:], in0=b0sm[:], scalar1=pi_col[:, 0:1],
                                    scalar2=None, op0=OP.mult)
            transpose_to(barrT, b0p, 128, 32)

            l0_ps = {}
            for i in range(nb + 1):
                if i < nb:
                    u0p = psp.tile([128, 128], fp32,
                                   tag="bx" if i % 2 else "qa", space="PSUM")
                    nc.tensor.matmul(out=u0p[:],
                                     lhsT=ohxt[:, i * 128:(i + 1) * 128],
                                     rhs=barrT[:], start=True, stop=True)
                    l0_ps[i] = u0p
                if i >= 1:
                    b = i - 1
                    ps = l0_ps.pop(b)
                    Z = sbp.tile([128, G], fp32, tag="Z")
                    nc.vector.reduce_sum(out=Z[:], in_=ps[:].rearrange(
                        "p (g c) -> p g c", c=C), axis=AX)
                    nc.scalar.activation(out=out_sb[:, b * 64:b * 64 + G],
                                         in_=Z[:], func=AF.Ln)
                    emit_h(0, b, ps[:], Z)

            # ================= graph layers =================
            for l in range(1, L):
                lq = l - 1

                qraw = sbp.tile([128, C], fp32, tag="qraw")
                nc.sync.dma_start(out=qraw[:], in_=lam_Q[lq])
                qsm = softmax_free(qraw, C, "q")  # [(g k), c]
                qsm_ap = qsm[:]
                qsm_bc = bass.AP(qsm_ap.tensor, qsm_ap.offset,
                                 [qsm_ap.ap[0], [0, G], qsm_ap.ap[1]])
                nc.vector.tensor_tensor(
                    out=qbig[:].rearrange("p (g c) -> p g c", c=C),
                    in0=qsm_bc,
                    in1=maskg[:].rearrange("p (g c) -> p g c", c=C),
                    op=OP.mult)
                braw2 = sbp.tile([128, M], fp32, tag="braw")
                nc.sync.dma_start(out=braw2[:], in_=lam_B[lq])
                bsm = softmax_free(braw2, M, "b")
                transpose_to(barrT, bsm, 128, 32)

                # ---- chunk caches (fresh per layer)
                gat_cache = {}
                oh_cache = {}

                def get_gat(gt, lq=lq):
                    ci = int(tile2chunk[gt])
                    if ci not in gat_cache:
                        k, t0, ntile = chunks[ci]
                        buf = gp.tile([128, ntile * 128], bf16, tag="g")
                        nc.gpsimd.dma_gather(
                            out_ap=buf[:].rearrange("p (t e) -> p t e", e=128),
                            in_ap=h_full[lq][k][:],
                            idxs_ap=idx_t[:, t0 * 8:(t0 + ntile) * 8],
                            num_idxs=ntile * 128,
                            num_idxs_reg=nidx_regs[ntile * 128],
                            elem_size=128,
                            single_packet=False,
                            queue_num=ci % NQ)
                        gat_cache[ci] = buf
                    k, t0, ntile = chunks[ci]
                    return gat_cache[ci][:].rearrange(
                        "p (t e) -> p t e", e=128)[:, gt - t0, :]

                def get_oh(gt):
                    ci = int(tile2chunk[gt])
                    if ci not in oh_cache:
                        k, t0, ntile = chunks[ci]
                        buf = ohp.tile([128, ntile * 128], bf16, tag="oh")
                        nc.sync.dma_start(
                            out=buf[:], in_=ohx_d[:, t0 * 128:(t0 + ntile) * 128])
                        oh_cache[ci] = buf
                    k, t0, ntile = chunks[ci]
                    return oh_cache[ci][:, (gt - t0) * 128:(gt - t0 + 1) * 128]

                # ---- sweep banks 0..NBANKS-2 -> bf16 partial aggregates
                for k in range(NBANKS - 1):
                    for b in range(nb):
                        nt = int(T[b, k])
                        ps = psa.tile([128, 128], fp32, tag="agg", space="PSUM")
                        if k > 0:  # re-inject prior partial: I^T @ aggb = aggb
                            nc.tensor.matmul(out=ps[:], lhsT=ident_b[:],
                                             rhs=aggb[b][:], start=True,
                                             stop=False)
                        for t in range(nt):
                            gt = int(off[k, b]) + t
                            nc.tensor.matmul(out=ps[:], lhsT=get_gat(gt),
                                             rhs=get_oh(gt),
                                             start=(t == 0 and k == 0),
                                             stop=(t == nt - 1))
                        nc.scalar.copy(out=aggb[b][:], in_=ps[:])

                # ---- last bank: block-major, 3-stage pipelined finish
                kL = NBANKS - 1
                aggF_d = {}
                s1_d = {}
                for i in range(nb + 2):
                    if i < nb:
                        b = i
                        nt = int(T[b, kL])
                        ps = psa.tile([128, 128], fp32, tag="agg", space="PSUM")
                        # re-inject partial: I^T @ aggb = aggb
                        nc.tensor.matmul(out=ps[:], lhsT=ident_b[:],
                                         rhs=aggb[b][:], start=True, stop=False)
                        for t in range(nt):
                            gt = int(off[kL, b]) + t
                            nc.tensor.matmul(out=ps[:], lhsT=get_gat(gt),
                                             rhs=get_oh(gt),
                                             start=False, stop=(t == nt - 1))
                        aggF = sbp.tile([128, 128], bf16, tag="aggF")
                        nc.scalar.copy(out=aggF[:], in_=ps[:])
                        aggF_d[b] = aggF
                    if 1 <= i <= nb:
                        b = i - 1
                        qaT = psp.tile([128, 128], fp32, tag="qa", space="PSUM")
                        nc.tensor.matmul(out=qaT[:], lhsT=qbig[:],
                                         rhs=aggF_d.pop(b)[:],
                                         start=True, stop=True)
                        qaTsb = sbp.tile([128, 128], fp32, tag="qaTsb")
                        nc.scalar.copy(out=qaTsb[:], in_=qaT[:])
                        bx = psp.tile([128, 128], fp32, tag="bx", space="PSUM")
                        nc.tensor.matmul(out=bx[:],
                                         lhsT=ohxt[:, b * 128:(b + 1) * 128],
                                         rhs=barrT[:], start=True, stop=True)
                        bxsb = sbp.tile([128, 128], fp32, tag="bxsb")
                        nc.scalar.copy(out=bxsb[:], in_=bx[:])
                        s1_d[b] = (qaTsb, bxsb)
                    if 2 <= i:
                        b = i - 2
                        qaTsb, bxsb = s1_d.pop(b)
                        qa2 = psp.tile([128, 128], fp32, tag="trp", space="PSUM")
                        nc.tensor.transpose(out=qa2[:], in_=qaTsb[:],
                                            identity=ident_f[:])
                        u = sbp.tile([128, 128], fp32, tag="u")
                        nc.vector.tensor_tensor(out=u[:], in0=qa2[:], in1=bxsb[:],
                                                op=OP.mult)
                        Z = sbp.tile([128, G], fp32, tag="Z")
                        nc.vector.reduce_sum(out=Z[:], in_=u[:].rearrange(
                            "p (g c) -> p g c", c=C), axis=AX)
                        nc.scalar.activation(
                            out=out_sb[:, b * 64 + l * G:b * 64 + (l + 1) * G],
                            in_=Z[:], func=AF.Ln)
                        if l < L - 1:
                            emit_h(l, b, u[:], Z)

            # ---- write lls out
            if nb > 1:
                nc.sync.dma_start(
                    out=lls_d[:(nb - 1) * 128, :].rearrange(
                        "(b p) c -> p b c", p=128),
                    in_=out_sb[:].rearrange("p (b c) -> p b c", c=64)[:, :nb - 1, :])
            nc.sync.dma_start(
                out=lls_d[(nb - 1) * 128:, :],
                in_=out_sb[:last_nn, (nb - 1) * 64:nb * 64])

    nc.compile()
    return nc


# ---- entry point ------------------------------------------------------------

def kernel(x, edge_index, lambda_B0, lambda_Pi, lambda_Q, lambda_B):
    cfg = Cfg()
    cores, sched, lncnt = preprocess(x, edge_index, cfg)
    consts = make_consts()
    nc = build_nc(cfg, sched)

    from concourse.bass_utils import run_bass_kernel_spmd
    params = permute_params(lambda_B0, lambda_Pi, lambda_Q, lambda_B)
    in_maps = []
    for c in range(cfg.ncores):
        m = dict(cores[c])
        m.update(params)
        m.update({k: np.ascontiguousarray(v) for k, v in consts.items()})
        in_maps.append(m)

    res = run_bass_kernel_spmd(nc, in_maps, core_ids=list(range(cfg.ncores)))
    out = np.concatenate([res.results[c]["lls"] for c in range(cfg.ncores)],
                         axis=0).reshape(N, L, G).astype(np.float32)
    out[:, 1:, :] -= lncnt[:, None, None]
    return out
